# revision 12
# baseline (speedup 1.0000x reference)
"""Trainium2 Bass kernel for nn_ActorCriticRange (moe_routing).

Data-parallel over batch B=2048 across 8 NeuronCores (256 rows/core).
All parameters replicated; heavy host-side weight folding:
  - attention q-token is constant -> fold k-projection into a 512->4 logits
    matmul (wq_eff), drop its bias (softmax-invariant), fold v/o projections
    into a single post-attention matmul (Mvo).
  - all weights pre-transposed to [in, out], padded to 128-multiples, bf16.
Token-major activations on-device; feature-major matmul operands produced via
DMA transposes (bf16).

ACT-table discipline: the whole kernel uses ONLY the gelu_apprx_tanh table set
(gelu + tanh). exp() is computed as (1+tanh(x/2))/(1-tanh(x/2)); rsqrt for
LayerNorm via DVE magic-constant Newton iteration; divisions via the DVE
reciprocal instruction. This avoids ~2.7us ACT table reloads in the hot loop.
"""

import numpy as np
import ml_dtypes

# ---- problem constants (hardcoded per spec) ----
B = 2048
N_CORES = 8
NTOK = 64                      # tokens per batch element
E = 128                        # embed dim
I = 512                        # inner dim
H = 4                          # heads
DH = I // H                    # 128
STATE = 48
MID = 320                      # kv-mlp mid dim
MIDP = 384                     # padded to 128-multiple
HID = 256
ACT = 12
M = 8                          # n_modules
FC = STATE + E                 # 176
FCP = 256                      # padded flat-ctx dim
MAGIC = 0x5F3759DF

BF16 = ml_dtypes.bfloat16

SIM_COMPOSED_GELU = False      # set True when running under CoreSim

_CACHE = {}


def _np(x):
    return np.asarray(x)


def _bf(x):
    return np.ascontiguousarray(np.asarray(x, dtype=np.float32).astype(BF16))


def _f32(x):
    return np.ascontiguousarray(np.asarray(x, dtype=np.float32))


def _pad_to(x, shape):
    out = np.zeros(shape, dtype=np.float32)
    out[tuple(slice(0, s) for s in x.shape)] = x
    return out


def _bcast_rows(v, p=128):
    v = np.asarray(v, dtype=np.float32)
    return np.broadcast_to(v[None, :], (p,) + v.shape).copy()


def _prep_weights(params):
    """Fold/transpose all parameters into device-layout numpy arrays."""
    p = params
    dev = {}

    def fuse_prep(tag, kvp, qtok, mha):
        W1 = _np(kvp["l1"]["W"])   # [320, 128]
        b1 = _np(kvp["l1"]["b"])
        W2 = _np(kvp["l2"]["W"])   # [512, 320]
        b2 = _np(kvp["l2"]["b"])
        Wq, bq = _np(mha["q"]["W"]), _np(mha["q"]["b"])
        Wk = _np(mha["k"]["W"])
        Wv, bv = _np(mha["v"]["W"]), _np(mha["v"]["b"])
        Wo, bo = _np(mha["o"]["W"]), _np(mha["o"]["b"])
        qt = _np(qtok)[0]

        for k in ("ln1", "ln2"):
            g, be = _np(kvp[k]["g"]), _np(kvp[k]["b"])
            assert np.allclose(g, 1.0) and np.allclose(be, 0.0), "non-trivial LN"

        dev[f"W1T_{tag}"] = _bf(_pad_to(W1.T, (E, MIDP)))
        dev[f"b1b_{tag}"] = _bf(_bcast_rows(_pad_to(b1, (MIDP,))))
        dev[f"W2T_{tag}"] = _bf(_pad_to(W2.T, (MIDP, I)).reshape(3, 128, I).transpose(1, 0, 2))
        dev[f"b2b_{tag}"] = _bf(_bcast_rows(b2))

        qh = (qt @ Wq.T + bq).reshape(H, DH)
        wq_eff = np.einsum("hd,hdi->hi", qh, Wk.reshape(H, DH, I)) / np.sqrt(DH)
        dev[f"wq_{tag}"] = _bf(np.ascontiguousarray(
            wq_eff.T.reshape(4, 128, H).transpose(1, 0, 2)))

        Mvo = np.concatenate(
            [Wv[h * DH:(h + 1) * DH, :].T @ Wo[:, h * DH:(h + 1) * DH].T
             for h in range(H)], axis=0)
        dev[f"Mvo_{tag}"] = _bf(Mvo.reshape(16, 128, I).transpose(1, 0, 2))
        dev[f"bob_{tag}"] = _bf(_bcast_rows(bv @ Wo.T + bo))

    fuse_prep("mod", p["kv_mod"], p["q_mod"], p["mhca_mod"])
    fuse_prep("sub", p["kv_sub"], p["q_sub"], p["mhca_sub"])

    pp = p["proj_mod"]
    g, be = _np(pp["ln1"]["g"]), _np(pp["ln1"]["b"])
    assert np.allclose(g, 1.0) and np.allclose(be, 0.0)
    dev["P1T"] = _bf(_pad_to(_np(pp["l1"]["W"]).T, (I, MIDP)).reshape(4, 128, MIDP).transpose(1, 0, 2))
    dev["pb1b"] = _bf(_bcast_rows(_pad_to(_np(pp["l1"]["b"]), (MIDP,))))
    dev["P2T"] = _bf(_pad_to(_np(pp["l2"]["W"]).T, (MIDP, E)).reshape(3, 128, E).transpose(1, 0, 2))
    dev["pb2b"] = _bf(_bcast_rows(_np(pp["l2"]["b"])))

    g = p["gate"]
    for k in ("bbln1", "bbln2"):
        gg, bb = _np(g[k]["g"]), _np(g[k]["b"])
        assert np.allclose(gg, 1.0) and np.allclose(bb, 0.0)
    dev["G1T"] = _bf(_pad_to(_np(g["bb1"]["W"]).T, (FCP, I)).reshape(2, 128, I).transpose(1, 0, 2))
    dev["gb1b"] = _bf(_bcast_rows(_np(g["bb1"]["b"])))
    dev["G2T"] = _bf(_np(g["bb2"]["W"]).T.reshape(4, 128, I).transpose(1, 0, 2))
    dev["gb2b"] = _bf(_bcast_rows(_np(g["bb2"]["b"])))
    dev["GlT"] = _bf(_np(g["logits"]["W"]).T.reshape(4, 128, 64).transpose(1, 0, 2))
    dev["glbb"] = _bf(_bcast_rows(_np(g["logits"]["b"])))
    dev["GsT"] = _bf(_np(g["scale"]["W"]).T.reshape(4, 128, 1536).transpose(1, 0, 2))
    dev["gsbb"] = _bf(_bcast_rows(_np(g["scale"]["b"]) + 1.0))   # fold the "1 +"

    aW = [_np(w) for w in p["aW"]]
    ab = [_np(v) for v in p["ab"]]
    dims_in = [FC, HID, HID, HID]
    for l in range(4):
        w = aW[l]
        O = w.shape[1]
        wt = _pad_to(w.transpose(2, 0, 1).reshape(dims_in[l], M * O), (256, M * O))
        dev[f"aWT{l}"] = _bf(wt.reshape(2, 128, M * O).transpose(1, 0, 2))
        dev[f"ab{l}b"] = _bf(_bcast_rows(ab[l].reshape(M * O)))

    mask_half = np.zeros((128, 2), dtype=np.float32)
    mask_half[:64, 0] = 1.0
    mask_half[64:, 1] = 1.0
    dev["mask_half"] = _f32(mask_half)
    ones2 = np.zeros((2, 128), dtype=np.float32)
    ones2[0, :64] = 1.0
    ones2[1, 64:] = 1.0
    dev["ones2"] = _f32(ones2)
    mask8 = np.zeros((128, 8), dtype=np.float32)
    mask8[:64, 0:4] = 1.0
    mask8[64:, 4:8] = 1.0
    dev["mask8"] = _f32(mask8)
    return dev


_F32R_WEIGHTS = ("mask_half", "ones2")


def _weight_specs(dev):
    return {k: (list(v.shape),
                "float32r" if k in _F32R_WEIGHTS else str(v.dtype))
            for k, v in dev.items()}


def _build(nc_mod, b_core):
    """Build the Bass graph for one core processing b_core batch rows."""
    import concourse.mybir as mybir
    import concourse.tile as tile

    nc = nc_mod
    dt = mybir.dt
    Alu = mybir.AluOpType
    Af = mybir.ActivationFunctionType
    f32r = dt.float32r

    T = b_core * NTOK
    TM = min(2048, T)            # macro-tile tokens
    N_MT = T // TM
    ST = TM // 128               # token subtiles per macro-tile
    BT = (b_core + 127) // 128   # batch tiles
    BP = min(128, b_core)        # batch partition size

    oe = nc.declare_dram_parameter("obs_embed_bf16", [T, E], dt.bfloat16, isOutput=False)
    ost = nc.declare_dram_parameter("obs_state", [b_core, STATE], dt.float32, isOutput=False)
    W = {}
    _dtmap = {"bfloat16": dt.bfloat16, "float32": dt.float32,
              "float32r": dt.float32r}
    for name, (shape, dtype) in nc._weight_specs.items():
        W[name] = nc.declare_dram_parameter(name, shape, _dtmap[dtype], isOutput=False)
    out = nc.declare_dram_parameter("out", [b_core, ACT], dt.float32, isOutput=True)

    ctx_dram = {f: nc.dram_tensor(f"ctx_dram_{f}", [b_core, H * I], dt.bfloat16)
                for f in ("mod", "sub")}
    fc_dram = {f: nc.dram_tensor(f"fc_dram_{f}", [b_core, FCP], dt.bfloat16)
               for f in ("mod", "sub")}
    z_dram = [nc.dram_tensor(f"z_dram_{i}", [b_core, I], dt.bfloat16) for i in range(2)]
    xn_dram = [nc.dram_tensor(f"xn_dram_{i}", [b_core, HID], dt.bfloat16) for i in range(3)]

    with tile.TileContext(nc) as tc:
        import contextlib
        ctxmgr = contextlib.ExitStack()
        with ctxmgr:
            consts = ctxmgr.enter_context(tc.tile_pool(name="consts", bufs=1))
            wres = ctxmgr.enter_context(tc.tile_pool(name="wres", bufs=1))
            mtp = ctxmgr.enter_context(tc.tile_pool(name="mtp", bufs=1))
            mtp2 = ctxmgr.enter_context(tc.tile_pool(name="mtp2", bufs=2))
            tiles = ctxmgr.enter_context(tc.tile_pool(name="tiles", bufs=2))
            stats = ctxmgr.enter_context(tc.tile_pool(name="stats", bufs=4))
            btp = ctxmgr.enter_context(tc.tile_pool(name="btp", bufs=1))
            btw = ctxmgr.enter_context(tc.tile_pool(name="btw", bufs=1))
            psA = ctxmgr.enter_context(tc.tile_pool(name="psA", bufs=2, space="PSUM"))
            psB = ctxmgr.enter_context(tc.tile_pool(name="psB", bufs=2, space="PSUM"))
            psS = ctxmgr.enter_context(tc.tile_pool(name="psS", bufs=4, space="PSUM"))

            def load_w(pool, name, tag=None):
                spec = nc._weight_specs[name]
                d = _dtmap[spec[1]]
                t = pool.tile(spec[0], d, tag=tag or name, name=f"w_{name}")
                nc.sync.dma_start(out=t[:], in_=W[name][:])
                return t

            mask_half = load_w(consts, "mask_half")
            ones2 = load_w(consts, "ones2")
            mask8 = load_w(consts, "mask8")
            magic_t = consts.tile([128, 16], dt.int32, tag="magic")
            nc.vector.memset(magic_t[:], MAGIC)

            Wsb = {}
            for f in ("mod", "sub"):
                for base in ("W1T", "b1b", "W2T", "b2b", "wq", "bob"):
                    Wsb[f"{base}_{f}"] = load_w(wres, f"{base}_{f}")
            for base in ("P1T", "pb1b", "P2T", "pb2b", "G1T", "gb1b", "G2T",
                         "gb2b", "GlT", "glbb", "gsbb"):
                Wsb[base] = load_w(wres, base)

            def rsqrt_batch(ve_ap, n, rstd_ap):
                """rstd_ap[P, n] = (ve_ap)^-1/2 via magic seed + 2 Newton."""
                P = ve_ap.shape[0]
                y = stats.tile([128, 16], dt.float32, tag="nw_y")
                t1 = stats.tile([128, 16], dt.float32, tag="nw_t1")
                t2 = stats.tile([128, 16], dt.float32, tag="nw_t2")
                nc.vector.tensor_scalar(out=y[:P, :n].bitcast(dt.int32),
                                        in0=ve_ap.bitcast(dt.int32), scalar1=1,
                                        scalar2=None, op0=Alu.arith_shift_right)
                nc.vector.tensor_tensor(out=y[:P, :n].bitcast(dt.int32),
                                        in0=magic_t[:P, :n],
                                        in1=y[:P, :n].bitcast(dt.int32),
                                        op=Alu.subtract)
                cur = y
                for it in range(2):
                    dst = rstd_ap if it == 1 else y[:P, :n]
                    nc.vector.tensor_tensor(out=t1[:P, :n], in0=cur[:P, :n],
                                            in1=cur[:P, :n], op=Alu.mult)
                    nc.vector.tensor_tensor(out=t1[:P, :n], in0=t1[:P, :n],
                                            in1=ve_ap, op=Alu.mult)
                    nc.vector.tensor_scalar(out=t2[:P, :n], in0=t1[:P, :n],
                                            scalar1=-0.5, scalar2=1.5,
                                            op0=Alu.mult, op1=Alu.add)
                    nc.vector.tensor_tensor(out=dst, in0=cur[:P, :n],
                                            in1=t2[:P, :n], op=Alu.mult)

            def gelu_act(out_ap, in_ap, scale_ap, bias_ap):
                if not SIM_COMPOSED_GELU:
                    nc.scalar.activation(out=out_ap, in_=in_ap,
                                         func=Af.Gelu_apprx_tanh,
                                         bias=bias_ap, scale=scale_ap)
                    return
                # CoreSim-only composed gelu (tanh approx)
                P, Fw = in_ap.shape[0], in_ap.free_size()
                n = tiles.tile([128, 512], dt.float32, tag="cg_n")
                nc.scalar.activation(out=n[:P, :Fw], in_=in_ap, func=Af.Identity,
                                     bias=bias_ap, scale=scale_ap)
                q = tiles.tile([128, 512], dt.float32, tag="cg_q")
                nc.scalar.activation(out=q[:P, :Fw], in_=n[:P, :Fw], func=Af.Square)
                a = tiles.tile([128, 512], dt.float32, tag="cg_a")
                nc.vector.tensor_scalar(out=a[:P, :Fw], in0=q[:P, :Fw],
                                        scalar1=0.044715, scalar2=1.0,
                                        op0=Alu.mult, op1=Alu.add)
                nc.vector.tensor_tensor(out=a[:P, :Fw], in0=a[:P, :Fw],
                                        in1=n[:P, :Fw], op=Alu.mult)
                th = tiles.tile([128, 512], dt.float32, tag="cg_t")
                nc.scalar.activation(out=th[:P, :Fw], in_=a[:P, :Fw], func=Af.Tanh,
                                     scale=0.7978845608028654)
                nc.vector.tensor_scalar(out=th[:P, :Fw], in0=th[:P, :Fw],
                                        scalar1=1.0, scalar2=0.5,
                                        op0=Alu.add, op1=Alu.mult)
                nc.vector.tensor_tensor(out=out_ap, in0=th[:P, :Fw],
                                        in1=n[:P, :Fw], op=Alu.mult)

            def texp(out_ap, in_ap, P):
                """out = exp(in) via tanh; shapes [P, n] fp32, in-place safe."""
                n = in_ap.free_size()
                th = tiles.tile([128, 1024], dt.float32, tag="texp_t")
                nc.scalar.activation(out=th[:P, :n], in_=in_ap, func=Af.Tanh,
                                     scale=0.5)
                num = tiles.tile([128, 1024], dt.float32, tag="texp_n")
                nc.vector.tensor_scalar(out=num[:P, :n], in0=th[:P, :n],
                                        scalar1=1.0, scalar2=None, op0=Alu.add)
                den = tiles.tile([128, 1024], dt.float32, tag="texp_d")
                nc.vector.tensor_scalar(out=den[:P, :n], in0=th[:P, :n],
                                        scalar1=-1.0, scalar2=-1.0,
                                        op0=Alu.mult, op1=Alu.subtract)
                nc.vector.reciprocal(out=den[:P, :n], in_=den[:P, :n])
                nc.vector.tensor_tensor(out=out_ap, in0=num[:P, :n],
                                        in1=den[:P, :n], op=Alu.mult)

            def bias_only(psum_ap, bias_sb, out_sb):
                P = psum_ap.shape[0]
                nc.vector.scalar_tensor_tensor(
                    out=out_sb, in0=psum_ap, scalar=1.0, in1=bias_sb[:P],
                    op0=Alu.mult, op1=Alu.add)

            # =========== Stage A: kv pipeline ===========
            for mt in range(N_MT):
                t0 = mt * TM
                xT = mtp2.tile([128, TM], dt.bfloat16, tag="xT")
                nc.sync.dma_start_transpose(xT[:], oe[t0:t0 + TM, :])

                for f in ("mod", "sub"):
                    xb1 = mtp.tile([128, ST, MIDP], dt.bfloat16, tag="xb1")
                    mv1 = stats.tile([128, ST, 2], dt.float32, tag="mv1")
                    for st in range(ST):
                        ps1 = psA.tile([128, MIDP], dt.float32, tag="ps1",
                                       name=f"ps1_{mt}_{f}_{st}")
                        nc.tensor.matmul(ps1[:], lhsT=xT[:, st * 128:(st + 1) * 128],
                                         rhs=Wsb[f"W1T_{f}"][:], start=True, stop=True)
                        bias_only(ps1[:], Wsb[f"b1b_{f}"], xb1[:, st, :])
                        st6 = stats.tile([128, 6], dt.float32, tag="st6")
                        nc.vector.bn_stats(out=st6[:], in_=xb1[:, st, :MID])
                        nc.vector.bn_aggr(out=mv1[:, st, :], in_=st6[:])
                    ve1 = stats.tile([128, ST], dt.float32, tag="ve1")
                    nc.vector.tensor_scalar(out=ve1[:], in0=mv1[:, :, 1],
                                            scalar1=1e-5, scalar2=None, op0=Alu.add)
                    rstd1 = stats.tile([128, ST], dt.float32, tag="rstd1")
                    rsqrt_batch(ve1[:], ST, rstd1[:])
                    nmu1 = stats.tile([128, ST], dt.float32, tag="nmu1")
                    nc.vector.scalar_tensor_tensor(
                        out=nmu1[:], in0=mv1[:, :, 0], scalar=-1.0, in1=rstd1[:],
                        op0=Alu.mult, op1=Alu.mult)
                    h1T = mtp.tile([128, 3, TM], dt.bfloat16, tag="h1T")
                    for st in range(ST):
                        h1 = tiles.tile([128, MIDP], dt.bfloat16, tag="h1")
                        gelu_act(h1[:], xb1[:, st, :], rstd1[:, st:st + 1],
                                 nmu1[:, st:st + 1])
                        for c in range(3):
                            nc.sync.dma_start_transpose(
                                h1T[:, c, st * 128:(st + 1) * 128],
                                h1[:, c * 128:(c + 1) * 128])

                    xb2 = mtp.tile([128, ST, I], dt.bfloat16, tag="xb2")
                    mv2 = stats.tile([128, ST, 2], dt.float32, tag="mv2")
                    for st in range(ST):
                        ps2 = psB.tile([128, I], dt.float32, tag="ps2",
                                       name=f"ps2_{mt}_{f}_{st}")
                        for c in range(3):
                            nc.tensor.matmul(ps2[:], lhsT=h1T[:, c, st * 128:(st + 1) * 128],
                                             rhs=Wsb[f"W2T_{f}"][:, c, :],
                                             start=(c == 0), stop=(c == 2))
                        bias_only(ps2[:], Wsb[f"b2b_{f}"], xb2[:, st, :])
                        st6b = stats.tile([128, 6], dt.float32, tag="st6")
                        nc.vector.bn_stats(out=st6b[:], in_=xb2[:, st, :])
                        nc.vector.bn_aggr(out=mv2[:, st, :], in_=st6b[:])
                    ve2 = stats.tile([128, ST], dt.float32, tag="ve1")
                    nc.vector.tensor_scalar(out=ve2[:], in0=mv2[:, :, 1],
                                            scalar1=1e-5, scalar2=None, op0=Alu.add)
                    rstd2 = stats.tile([128, ST], dt.float32, tag="rstd1")
                    rsqrt_batch(ve2[:], ST, rstd2[:])
                    nmu2 = stats.tile([128, ST], dt.float32, tag="nmu1")
                    nc.vector.scalar_tensor_tensor(
                        out=nmu2[:], in0=mv2[:, :, 0], scalar=-1.0, in1=rstd2[:],
                        op0=Alu.mult, op1=Alu.mult)

                    kv_mt = mtp.tile([128, ST, I], dt.bfloat16, tag="kv")
                    tlg = mtp.tile([128, ST, H], dt.float32, tag="tlg")
                    S_mt = mtp.tile([2, ST, H], dt.float32r, tag="S_mt")
                    for st in range(ST):
                        gelu_act(kv_mt[:, st, :], xb2[:, st, :], rstd2[:, st:st + 1],
                                 nmu2[:, st:st + 1])
                        kvT = tiles.tile([128, 4, 128], dt.bfloat16, tag="kvT")
                        for c in range(4):
                            nc.sync.dma_start_transpose(
                                kvT[:, c, :], kv_mt[:, st, c * 128:(c + 1) * 128])
                        plg = psS.tile([128, H], dt.float32, tag="psm",
                                       name=f"plg_{mt}_{f}_{st}")
                        for c in range(4):
                            nc.tensor.matmul(plg[:], lhsT=kvT[:, c, :],
                                             rhs=Wsb[f"wq_{f}"][:, c, :],
                                             start=(c == 0), stop=(c == 3))
                        # tanh(logit/2) for the softmax exp
                        nc.scalar.activation(out=tlg[:, st, :], in_=plg[:],
                                             func=Af.Tanh, scale=0.5)
                    # exp = (1+t)/(1-t), batched over the macro-tile
                    elg = mtp.tile([128, ST, H], dt.float32r, tag="elg")
                    den = mtp.tile([128, ST, H], dt.float32, tag="den")
                    nc.vector.tensor_scalar(out=elg[:], in0=tlg[:], scalar1=1.0,
                                            scalar2=None, op0=Alu.add)
                    nc.vector.tensor_scalar(out=den[:], in0=tlg[:], scalar1=-1.0,
                                            scalar2=-1.0, op0=Alu.mult, op1=Alu.subtract)
                    nc.vector.reciprocal(out=den[:], in_=den[:])
                    nc.vector.tensor_tensor(out=elg[:], in0=elg[:].bitcast(dt.float32),
                                            in1=den[:], op=Alu.mult)
                    for st in range(ST):
                        pS = psS.tile([2, H], dt.float32, tag="psm",
                                      name=f"pS_{mt}_{f}_{st}")
                        nc.tensor.matmul(pS[:], lhsT=mask_half[:],
                                         rhs=elg[:, st, :],
                                         start=True, stop=True)
                        nc.vector.tensor_copy(out=S_mt[:, st, :], in_=pS[:])
                    pSb = psS.tile([128, ST * H], dt.float32, tag="psm",
                                   name=f"pSb_{mt}_{f}")
                    nc.tensor.matmul(pSb[:], lhsT=ones2[:],
                                     rhs=S_mt[:].rearrange("a b c -> a (b c)"),
                                     start=True, stop=True)
                    rS = mtp.tile([128, ST, H], dt.float32, tag="rS")
                    nc.vector.reciprocal(out=rS[:].rearrange("a b c -> a (b c)"),
                                         in_=pSb[:])
                    att = mtp.tile([128, ST, H], dt.float32, tag="att")
                    nc.vector.tensor_tensor(out=att[:], in0=elg[:].bitcast(dt.float32),
                                            in1=rS[:], op=Alu.mult)
                    att_bd = mtp.tile([128, ST, 2, H], dt.bfloat16, tag="attbd")
                    nc.vector.tensor_tensor(
                        out=att_bd[:],
                        in0=att[:, :, None, :].to_broadcast([128, ST, 2, H]),
                        in1=mask8[:].rearrange("p (a b) -> p a b", b=H)
                            .unsqueeze(1).to_broadcast([128, ST, 2, H]),
                        op=Alu.mult)
                    for st in range(ST):
                        pctx = psS.tile([8, I], dt.float32, tag="psm",
                                        name=f"pctx_{mt}_{f}_{st}")
                        nc.tensor.matmul(pctx[:],
                                         lhsT=att_bd[:, st, :, :].rearrange("p a b -> p (a b)"),
                                         rhs=kv_mt[:, st, :], start=True, stop=True)
                        ctx_sb = tiles.tile([8, I], dt.bfloat16, tag="ctxsb")
                        nc.vector.tensor_copy(out=ctx_sb[:], in_=pctx[:])
                        row = t0 // NTOK + st * 2
                        nc.sync.dma_start(
                            out=ctx_dram[f][row:row + 2, :].rearrange("b (h i) -> (b h) i", i=I),
                            in_=ctx_sb[:])

            # =========== Stage B: o = ctx @ Mvo; proj MLP ===========
            emb = {}
            for f in ("mod", "sub"):
                ctxT = btp.tile([128, 16, b_core], dt.bfloat16, tag="ctxT",
                                name=f"ctxT_{f}")
                for k in range(16):
                    nc.sync.dma_start_transpose(
                        ctxT[:, k, :], ctx_dram[f][:, k * 128:(k + 1) * 128])
                Mvo_sb = btw.tile([128, 16, I], dt.bfloat16, tag="bigw",
                                  name=f"Mvo_sb_{f}")
                nc.sync.dma_start(out=Mvo_sb[:], in_=W[f"Mvo_{f}"][:])
                oT = btp.tile([128, 4, b_core], dt.bfloat16, tag="oT",
                              name=f"oT_{f}")
                for bt in range(BT):
                    b0 = bt * 128
                    po = psB.tile([128, I], dt.float32, tag="ps2", name=f"po_{f}_{bt}")
                    for k in range(16):
                        nc.tensor.matmul(po[:BP], lhsT=ctxT[:, k, b0:b0 + BP],
                                         rhs=Mvo_sb[:, k, :], start=(k == 0), stop=(k == 15))
                    o_sb = tiles.tile([128, I], dt.bfloat16, tag="o_sb")
                    bias_only(po[:BP], Wsb[f"bob_{f}"], o_sb[:BP])
                    for c in range(4):
                        nc.sync.dma_start_transpose(
                            oT[:, c, b0:b0 + BP], o_sb[:BP, c * 128:(c + 1) * 128])
                hpT = btp.tile([128, 3, b_core], dt.bfloat16, tag="hpT",
                               name=f"hpT_{f}")
                for bt in range(BT):
                    b0 = bt * 128
                    pp1 = psA.tile([128, MIDP], dt.float32, tag="ps1", name=f"pp1_{f}_{bt}")
                    for c in range(4):
                        nc.tensor.matmul(pp1[:BP], lhsT=oT[:, c, b0:b0 + BP],
                                         rhs=Wsb["P1T"][:, c, :], start=(c == 0), stop=(c == 3))
                    xbp = tiles.tile([128, MIDP], dt.bfloat16, tag="h1", name=f"xbp_{f}_{bt}")
                    bias_only(pp1[:BP], Wsb["pb1b"], xbp[:BP])
                    st6p = stats.tile([128, 6], dt.float32, tag="st6")
                    mvp = stats.tile([128, 2], dt.float32, tag="mvp")
                    nc.vector.bn_stats(out=st6p[:BP], in_=xbp[:BP, :MID])
                    nc.vector.bn_aggr(out=mvp[:BP], in_=st6p[:BP])
                    vep = stats.tile([128, 1], dt.float32, tag="vep")
                    nc.vector.tensor_scalar(out=vep[:BP], in0=mvp[:BP, 1:2],
                                            scalar1=1e-5, scalar2=None, op0=Alu.add)
                    rstdp = stats.tile([128, 1], dt.float32, tag="rstdp")
                    rsqrt_batch(vep[:BP], 1, rstdp[:BP])
                    nmup = stats.tile([128, 1], dt.float32, tag="nmup")
                    nc.vector.scalar_tensor_tensor(
                        out=nmup[:BP], in0=mvp[:BP, 0:1], scalar=-1.0, in1=rstdp[:BP],
                        op0=Alu.mult, op1=Alu.mult)
                    hp = tiles.tile([128, MIDP], dt.bfloat16, tag="h1", name=f"hp_{f}_{bt}")
                    gelu_act(hp[:BP], xbp[:BP], rstdp[:BP], nmup[:BP])
                    for c in range(3):
                        nc.sync.dma_start_transpose(
                            hpT[:, c, b0:b0 + BP], hp[:BP, c * 128:(c + 1) * 128])
                emb_f = btp.tile([128, BT, E], dt.float32, tag=f"emb_{f}",
                                 name=f"emb_{f}")
                for bt in range(BT):
                    b0 = bt * 128
                    pp2 = psS.tile([128, E], dt.float32, tag="psm", name=f"pp2_{f}_{bt}")
                    for c in range(3):
                        nc.tensor.matmul(pp2[:BP], lhsT=hpT[:, c, b0:b0 + BP],
                                         rhs=Wsb["P2T"][:, c, :], start=(c == 0), stop=(c == 2))
                    bias_only(pp2[:BP], Wsb["pb2b"], emb_f[:BP, bt, :])
                emb[f] = emb_f

            # =========== Stage C: flat vectors ===========
            fcT = {}
            for f in ("mod", "sub"):
                fcT_f = btp.tile([128, 2, b_core], dt.bfloat16, tag=f"fcT_{f}",
                                 name=f"fcT_{f}")
                for bt in range(BT):
                    b0 = bt * 128
                    fcb = tiles.tile([128, FCP], dt.bfloat16, tag="fcb")
                    nc.vector.memset(fcb[:], 0.0)
                    ost_sb = tiles.tile([128, STATE], dt.float32, tag="ost_sb")
                    nc.sync.dma_start(out=ost_sb[:BP], in_=ost[b0:b0 + BP, :])
                    nc.vector.tensor_copy(out=fcb[:BP, 0:STATE], in_=ost_sb[:BP])
                    nc.vector.tensor_copy(out=fcb[:BP, STATE:FC], in_=emb[f][:BP, bt, :])
                    nc.sync.dma_start(out=fc_dram[f][b0:b0 + BP, :], in_=fcb[:BP])
                for c in range(2):
                    nc.sync.dma_start_transpose(
                        fcT_f[:, c, :], fc_dram[f][:, c * 128:(c + 1) * 128])
                fcT[f] = fcT_f

            # =========== Stage D: gate backbone ===========
            def ln_gelu_bt(psum_ap, bias_name, out_sb, width):
                P = psum_ap.shape[0]
                xbz = tiles.tile([128, I], dt.bfloat16, tag="o_sb", name=f"xbz_{bias_name}")
                bias_only(psum_ap, Wsb[bias_name], xbz[:P, :width])
                st6z = stats.tile([128, 6], dt.float32, tag="st6")
                mvz = stats.tile([128, 2], dt.float32, tag="mvp")
                nc.vector.bn_stats(out=st6z[:P], in_=xbz[:P, :width])
                nc.vector.bn_aggr(out=mvz[:P], in_=st6z[:P])
                vez = stats.tile([128, 1], dt.float32, tag="vep")
                nc.vector.tensor_scalar(out=vez[:P], in0=mvz[:P, 1:2],
                                        scalar1=1e-5, scalar2=None, op0=Alu.add)
                rstdz = stats.tile([128, 1], dt.float32, tag="rstdp")
                rsqrt_batch(vez[:P], 1, rstdz[:P])
                nmuz = stats.tile([128, 1], dt.float32, tag="nmup")
                nc.vector.scalar_tensor_tensor(
                    out=nmuz[:P], in0=mvz[:P, 0:1], scalar=-1.0, in1=rstdz[:P],
                    op0=Alu.mult, op1=Alu.mult)
                gelu_act(out_sb, xbz[:P, :width], rstdz[:P], nmuz[:P])

            zT = btp.tile([128, 4, b_core], dt.bfloat16, tag="zT")
            z2T = btp.tile([128, 4, b_core], dt.bfloat16, tag="zT", name="z2T")
            for bt in range(BT):
                b0 = bt * 128
                pz = psB.tile([128, I], dt.float32, tag="ps2", name=f"pz1_{bt}")
                for c in range(2):
                    nc.tensor.matmul(pz[:BP], lhsT=fcT["mod"][:, c, b0:b0 + BP],
                                     rhs=Wsb["G1T"][:, c, :], start=(c == 0), stop=(c == 1))
                z1 = tiles.tile([128, I], dt.bfloat16, tag="o_sb", name=f"z1_{bt}")
                ln_gelu_bt(pz[:BP], "gb1b", z1[:BP], I)
                nc.sync.dma_start(out=z_dram[0][b0:b0 + BP, :], in_=z1[:BP])
            for c in range(4):
                nc.sync.dma_start_transpose(zT[:, c, :], z_dram[0][:, c * 128:(c + 1) * 128])
            for bt in range(BT):
                b0 = bt * 128
                pz = psB.tile([128, I], dt.float32, tag="ps2", name=f"pz2_{bt}")
                for c in range(4):
                    nc.tensor.matmul(pz[:BP], lhsT=zT[:, c, b0:b0 + BP],
                                     rhs=Wsb["G2T"][:, c, :], start=(c == 0), stop=(c == 3))
                z2 = tiles.tile([128, I], dt.bfloat16, tag="o_sb", name=f"z2_{bt}")
                ln_gelu_bt(pz[:BP], "gb2b", z2[:BP], I)
                nc.sync.dma_start(out=z_dram[1][b0:b0 + BP, :], in_=z2[:BP])
            for c in range(4):
                nc.sync.dma_start_transpose(z2T[:, c, :], z_dram[1][:, c * 128:(c + 1) * 128])

            pg = btp.tile([128, BT, M, M], dt.float32, tag="pg")
            sc = btp.tile([128, BT, 3, I], dt.float32, tag="sc")
            GsT_sb = btw.tile([128, 4, 1536], dt.bfloat16, tag="bigw", name="GsT_sb")
            nc.sync.dma_start(out=GsT_sb[:], in_=W["GsT"][:])
            for bt in range(BT):
                b0 = bt * 128
                pl = psS.tile([128, 64], dt.float32, tag="psm", name=f"pl_{bt}")
                for c in range(4):
                    nc.tensor.matmul(pl[:BP], lhsT=z2T[:, c, b0:b0 + BP],
                                     rhs=Wsb["GlT"][:, c, :], start=(c == 0), stop=(c == 3))
                pgl = tiles.tile([128, 64], dt.float32, tag="pgl")
                bias_only(pl[:BP], Wsb["glbb"], pgl[:BP])
                e_pg = tiles.tile([128, M, M], dt.float32, tag="epg")
                texp(e_pg[:BP].rearrange("p a b -> p (a b)"), pgl[:BP], BP)
                pgs = stats.tile([128, M], dt.float32, tag="pgs")
                for gidx in range(M):
                    nc.vector.reduce_sum(out=pgs[:BP, gidx:gidx + 1],
                                         in_=e_pg[:BP, gidx, :],
                                         axis=mybir.AxisListType.X)
                nc.vector.reciprocal(out=pgs[:BP], in_=pgs[:BP])
                nc.vector.tensor_tensor(
                    out=pg[:BP, bt], in0=e_pg[:BP],
                    in1=pgs[:BP, :, None].to_broadcast([BP, M, M]), op=Alu.mult)
                for j in range(3):
                    psc = psB.tile([128, I], dt.float32, tag="ps2", name=f"psc_{bt}_{j}")
                    for c in range(4):
                        nc.tensor.matmul(psc[:BP], lhsT=z2T[:, c, b0:b0 + BP],
                                         rhs=GsT_sb[:, c, j * I:(j + 1) * I],
                                         start=(c == 0), stop=(c == 3))
                    bias_only(psc[:BP], Wsb["gsbb"][:, j * I:(j + 1) * I],
                              sc[:BP, bt, j, :])

            # =========== Stage E: MoE ===========
            xcurT = fcT["sub"]
            final = btp.tile([128, BT, ACT], dt.float32, tag="final")
            for l in range(4):
                O = ACT if l == 3 else HID
                aw = btw.tile([128, 2, M * O], dt.bfloat16, tag="bigw", name=f"aw{l}")
                nc.sync.dma_start(out=aw[:], in_=W[f"aWT{l}"][:])
                abb = btw.tile([128, M * O], dt.bfloat16, tag="abb", name=f"abb{l}")
                nc.sync.dma_start(out=abb[:], in_=W[f"ab{l}b"][:])
                if l < 3:
                    nxT = btp.tile([128, 2, b_core], dt.bfloat16,
                                   tag=f"nxT{l % 2}", name=f"nxT{l}")
                else:
                    nxT = None
                for bt in range(BT):
                    b0 = bt * 128
                    nj = (M * O + I - 1) // I
                    phs = []
                    for j in range(nj):
                        w_ = min(I, M * O - j * I)
                        if w_ == I:
                            ph = psB.tile([128, I], dt.float32, tag="ps2",
                                          name=f"ph{l}_{bt}_{j}")
                        else:
                            ph = psS.tile([128, w_], dt.float32, tag="psm",
                                          name=f"ph{l}_{bt}_{j}")
                        for c in range(2):
                            nc.tensor.matmul(ph[:BP, :w_], lhsT=xcurT[:, c, b0:b0 + BP],
                                             rhs=aw[:, c, j * I:j * I + w_],
                                             start=(c == 0), stop=(c == 1))
                        phs.append(ph)

                    def h_m(m):
                        j = (m * O) // I
                        off = m * O - j * I
                        return phs[j][:BP, off:off + O]

                    acc = tiles.tile([128, O], dt.float32, tag=f"acc{O}")
                    acc2 = tiles.tile([128, O], dt.float32, tag=f"acc2{O}")
                    gw = pg[:BP, bt, 2 * l, :]
                    gb = pg[:BP, bt, 2 * l + 1, :]
                    nc.vector.tensor_scalar_mul(out=acc[:BP], in0=h_m(0),
                                                scalar1=gw[:, 0:1])
                    for m in range(1, M):
                        nc.vector.scalar_tensor_tensor(
                            out=acc[:BP], in0=h_m(m), scalar=gw[:, m:m + 1],
                            in1=acc[:BP], op0=Alu.mult, op1=Alu.add)
                    nc.vector.tensor_scalar_mul(out=acc2[:BP], in0=abb[:BP, 0:O],
                                                scalar1=gb[:, 0:1])
                    for m in range(1, M):
                        nc.vector.scalar_tensor_tensor(
                            out=acc2[:BP], in0=abb[:BP, m * O:(m + 1) * O],
                            scalar=gb[:, m:m + 1], in1=acc2[:BP],
                            op0=Alu.mult, op1=Alu.add)
                    if l == 3:
                        nc.vector.tensor_add(out=final[:BP, bt, :], in0=acc[:BP],
                                             in1=acc2[:BP])
                    else:
                        scf = sc[:BP, bt].rearrange("p a b -> p (a b)")
                        u = tiles.tile([128, O], dt.float32, tag=f"u{O}")
                        nc.vector.tensor_tensor(out=u[:BP], in0=acc[:BP],
                                                in1=scf[:, 2 * l * HID:(2 * l + 1) * HID],
                                                op=Alu.mult)
                        v = tiles.tile([128, O], dt.float32, tag=f"v{O}")
                        nc.vector.tensor_tensor(out=v[:BP], in0=acc2[:BP],
                                                in1=scf[:, (2 * l + 1) * HID:(2 * l + 2) * HID],
                                                op=Alu.mult)
                        w_sum = tiles.tile([128, O], dt.float32, tag=f"w{O}")
                        nc.vector.tensor_add(out=w_sum[:BP], in0=u[:BP], in1=v[:BP])
                        # ELU = max(w,0) + exp(min(w,0)) - 1
                        neg = tiles.tile([128, O], dt.float32, tag=f"neg{O}")
                        nc.vector.tensor_scalar(out=neg[:BP], in0=w_sum[:BP],
                                                scalar1=0.0, scalar2=None, op0=Alu.min)
                        ex = tiles.tile([128, O], dt.float32, tag=f"ex{O}")
                        texp(ex[:BP], neg[:BP], BP)
                        pos = tiles.tile([128, O], dt.float32, tag=f"pos{O}")
                        nc.vector.tensor_scalar(out=pos[:BP], in0=w_sum[:BP],
                                                scalar1=0.0, scalar2=None, op0=Alu.max)
                        xn = tiles.tile([128, O], dt.bfloat16, tag=f"xn{O}")
                        nc.vector.scalar_tensor_tensor(
                            out=xn[:BP], in0=pos[:BP], scalar=-1.0, in1=ex[:BP],
                            op0=Alu.add, op1=Alu.add)
                        nc.sync.dma_start(out=xn_dram[l][b0:b0 + BP, :], in_=xn[:BP])
                if l < 3:
                    for c in range(2):
                        nc.sync.dma_start_transpose(
                            nxT[:, c, :], xn_dram[l][:, c * 128:(c + 1) * 128])
                    xcurT = nxT

            for bt in range(BT):
                b0 = bt * 128
                nc.sync.dma_start(out=out[b0:b0 + BP, :], in_=final[:BP, bt, :])

    return nc


def _get_compiled(b_core):
    key = ("nc", b_core)
    if key in _CACHE:
        return _CACHE[key]
    from concourse import bacc
    nc = bacc.Bacc("TRN2", target_bir_lowering=False, debug=False,
                   num_devices=N_CORES)
    nc._weight_specs = _CACHE["wspecs"]
    _build(nc, b_core)
    nc.compile()
    _CACHE[key] = nc
    return nc


def kernel(obs_state, obs_embed, params):
    obs_state = np.asarray(obs_state, dtype=np.float32)
    obs_embed = np.asarray(obs_embed, dtype=np.float32)
    dev = _prep_weights(params)
    _CACHE["wspecs"] = _weight_specs(dev)
    b_core = obs_state.shape[0] // N_CORES
    nc = _get_compiled(b_core)

    in_maps = []
    for c in range(N_CORES):
        sl = slice(c * b_core, (c + 1) * b_core)
        m = dict(dev)
        m["obs_state"] = np.ascontiguousarray(obs_state[sl])
        m["obs_embed_bf16"] = np.ascontiguousarray(
            obs_embed[sl].reshape(b_core * NTOK, E).astype(BF16))
        in_maps.append(m)

    from concourse.bass_utils import run_bass_kernel_spmd
    res = run_bass_kernel_spmd(nc, in_maps, core_ids=list(range(N_CORES)))
    out = np.concatenate([res.results[c]["out"] for c in range(N_CORES)], axis=0)
    return out.astype(np.float32)
